# revision 4
# baseline (speedup 1.0000x reference)
"""Trainium2 Bass kernel for CrossModalAttention2D.

Math note: the attention has kv_len == 1 (text is mean-pooled to a single
token), so softmax over the key axis is identically 1.0 and the attention
output for every query position equals v[b].  The LayerNorm + Q projection
therefore do not affect the output at all; the module reduces exactly to

    t[b]   = mean_n text[b, n, :]                      # (C,)
    p[b]   = ((t Wv^T + bv) out_w^T + out_b) proj_w^T + proj_b
    out    = visual + gamma * p[b][None, :, None, None]

which is what this kernel computes (verified to 5e-8 normalized rel err
against the full reference).  Sharding: data-parallel over B — core c
handles batch c.  Weights (3 x 1024x1024) are replicated to every core and
the small chain is recomputed per core; the 16 MB visual slice per core
dominates traffic.
"""

import os
import sys

sys.path.insert(0, "/opt/trn_rl_repo")

import numpy as np

import concourse.bass as bass
import concourse.mybir as mybir
from concourse.tile import TileContext
from concourse.bass_utils import run_bass_kernel_spmd

B, C, H, W, NH, NT = 8, 1024, 64, 64, 16, 8
HW = H * W
P = 128
NCH = C // P  # 8 channel chunks
F32 = mybir.dt.float32


def _split_waits(nc):
    """walrus in this env accepts at most ONE sync-wait per instruction.
    Hoist extra waits onto NoOps inserted just before, on the same engine
    (per-engine program order makes this semantically identical)."""
    for fn in nc.m.functions:
        for blk in fn.blocks:
            rebuilt = []
            changed = False
            for inst in blk.instructions:
                si = inst.sync_info
                if si is not None and si.on_wait is not None and len(si.on_wait) > 1:
                    waits = list(si.on_wait)
                    for i, w in enumerate(waits[:-1]):
                        rebuilt.append(
                            mybir.InstNoOp(
                                name=f"{inst.name}-ws{i}",
                                engine=inst.engine,
                                sync_info=mybir.SyncInfo(on_wait=[w], on_update=[]),
                                bass_nofuse=True,
                            )
                        )
                    si.on_wait = [waits[-1]]
                    changed = True
                rebuilt.append(inst)
            if changed:
                blk.instructions = rebuilt


def _build_nc(vis_bufs=8, w_bufs=4):
    nc = bass.Bass(trn_type="TRN2")

    vis = nc.dram_tensor("vis", [C, HW], F32, kind="ExternalInput")
    textT = nc.dram_tensor("textT", [C, NT], F32, kind="ExternalInput")
    wvT = nc.dram_tensor("wvT", [C, C], F32, kind="ExternalInput")
    woT = nc.dram_tensor("woT", [C, C], F32, kind="ExternalInput")
    wpT = nc.dram_tensor("wpT", [C, C], F32, kind="ExternalInput")
    bv_col = nc.dram_tensor("bv_col", [P, NCH], F32, kind="ExternalInput")
    bo_col = nc.dram_tensor("bo_col", [P, NCH], F32, kind="ExternalInput")
    bp_col = nc.dram_tensor("bp_col", [P, NCH], F32, kind="ExternalInput")
    gamma_col = nc.dram_tensor("gamma_col", [P, 1], F32, kind="ExternalInput")
    out = nc.dram_tensor("out", [C, HW], F32, kind="ExternalOutput")

    with TileContext(nc) as tc:
        with (
            tc.tile_pool(name="cols", bufs=1) as cols,
            tc.tile_pool(name="wpool", bufs=w_bufs) as wpool,
            tc.tile_pool(name="psum", bufs=NCH, space="PSUM") as psum,
            tc.tile_pool(name="vispool", bufs=vis_bufs) as vispool,
        ):
            # ---- constants ----
            bvc = cols.tile([P, NCH], F32, tag="bvc")
            boc = cols.tile([P, NCH], F32, tag="boc")
            bpc = cols.tile([P, NCH], F32, tag="bpc")
            gc = cols.tile([P, 1], F32, tag="gc")
            nc.sync.dma_start(out=bvc, in_=bv_col[:, :])
            nc.sync.dma_start(out=boc, in_=bo_col[:, :])
            nc.sync.dma_start(out=bpc, in_=bp_col[:, :])
            nc.sync.dma_start(out=gc, in_=gamma_col[:, :])

            # ---- t = mean(text) in column layout: NCH tiles of (P, 1) ----
            tcols = []
            for k in range(NCH):
                tt = cols.tile([P, NT], F32, tag=f"tt{k}")
                nc.sync.dma_start(out=tt, in_=textT[k * P : (k + 1) * P, :])
                tk = cols.tile([P, 1], F32, tag=f"t{k}")
                nc.vector.reduce_sum(tk, tt, axis=mybir.AxisListType.X)
                nc.vector.tensor_scalar_mul(tk, tk, 1.0 / NT)
                tcols.append(tk)

            # ---- 3-layer chain: each layer out_col[mo] = sum_k WT[k-chunk, mo-chunk]^T @ in_col[k] ----
            def layer(in_cols, wT_dram, bias_tile, name, gamma_tile=None):
                ps = [psum.tile([P, 1], F32, tag="ps", name=f"ps_{name}{i}")
                      for i in range(NCH)]
                for k in range(NCH):
                    wt = wpool.tile([P, C], F32, tag="wt")
                    nc.sync.dma_start(out=wt, in_=wT_dram[k * P : (k + 1) * P, :])
                    for mo in range(NCH):
                        nc.tensor.matmul(
                            ps[mo],
                            wt[:, mo * P : (mo + 1) * P],
                            in_cols[k],
                            start=(k == 0),
                            stop=(k == NCH - 1),
                        )
                outs = []
                for mo in range(NCH):
                    oc = cols.tile([P, 1], F32, tag=f"{name}{mo}")
                    if gamma_tile is None:
                        nc.vector.tensor_tensor(
                            out=oc, in0=ps[mo], in1=bias_tile[:, mo : mo + 1],
                            op=mybir.AluOpType.add,
                        )
                    else:
                        # oc = (psum + bias) * gamma
                        nc.vector.tensor_scalar(
                            oc, ps[mo],
                            bias_tile[:, mo : mo + 1], gamma_tile[:, 0:1],
                            op0=mybir.AluOpType.add, op1=mybir.AluOpType.mult,
                        )
                    outs.append(oc)
                return outs

            vcols = layer(tcols, wvT, bvc, "v")
            ucols = layer(vcols, woT, boc, "u")
            gpcols = layer(ucols, wpT, bpc, "g", gamma_tile=gc)

            # ---- main loop: out = vis + gp (broadcast along free dim) ----
            for m in range(NCH):
                vt = vispool.tile([P, HW], F32, tag="vt")
                nc.sync.dma_start(out=vt, in_=vis[m * P : (m + 1) * P, :])
                nc.vector.tensor_scalar_add(vt, vt, gpcols[m][:, 0:1])
                nc.sync.dma_start(out=out[m * P : (m + 1) * P, :], in_=vt)

    _split_waits(nc)
    return nc


def _install_ntff_hook():
    """This container's `antenv` stub lacks axon_hooks; recreate the NTFF
    profiling hook via ctypes against the axon PJRT .so (same logic as
    trn_agent_boot.trn_boot)."""
    try:
        from antenv.axon_hooks import get_axon_ntff_profile_hook  # noqa: F401
        return
    except ImportError:
        pass
    import contextlib
    import ctypes
    import types

    so_path = "/opt/axon/libaxon_pjrt.so"
    if not os.path.exists(so_path):
        return
    lib = ctypes.CDLL(so_path)
    if not hasattr(lib, "axon_start_nrt_profile"):
        return
    lib.axon_start_nrt_profile.argtypes = [
        ctypes.POINTER(ctypes.c_int64), ctypes.c_size_t,
    ]
    lib.axon_start_nrt_profile.restype = ctypes.c_int64
    lib.axon_stop_nrt_profile.argtypes = [ctypes.c_char_p]
    lib.axon_stop_nrt_profile.restype = ctypes.c_int64

    @contextlib.contextmanager
    def _hook(output_dir, device_ids):
        import jax

        jax.devices()
        if device_ids:
            ids = (ctypes.c_int64 * len(device_ids))(*device_ids)
            rc = lib.axon_start_nrt_profile(ids, len(device_ids))
        else:
            rc = lib.axon_start_nrt_profile(None, 0)
        if rc != 0:
            raise RuntimeError(f"axon_start_nrt_profile rc={rc}")
        try:
            yield
        finally:
            n = lib.axon_stop_nrt_profile(str(output_dir).encode())
            print(f"ntff profile: {n} file(s) written to {output_dir}")

    import antenv

    mod = types.ModuleType("antenv.axon_hooks")
    mod.get_axon_ntff_profile_hook = lambda: _hook
    mod.set_axon_ntff_profile_hook = lambda h: None
    sys.modules["antenv.axon_hooks"] = mod
    antenv.axon_hooks = mod


_NC_CACHE = {}


def _get_nc():
    if "nc" not in _NC_CACHE:
        _NC_CACHE["nc"] = _build_nc()
    return _NC_CACHE["nc"]


def kernel(visual, text, in_proj_w, in_proj_b, out_w, out_b, ln_w, ln_b,
           proj_w, proj_b, gamma):
    visual = np.ascontiguousarray(np.asarray(visual, dtype=np.float32))
    text = np.asarray(text, dtype=np.float32)
    in_proj_w = np.asarray(in_proj_w, dtype=np.float32)
    in_proj_b = np.asarray(in_proj_b, dtype=np.float32)

    # host-side input marshalling (layout only, no math)
    wvT = np.ascontiguousarray(in_proj_w[2 * C : 3 * C].T)   # [c, j]
    woT = np.ascontiguousarray(np.asarray(out_w, dtype=np.float32).T)
    wpT = np.ascontiguousarray(np.asarray(proj_w, dtype=np.float32).T)
    bv_col = np.ascontiguousarray(in_proj_b[2 * C : 3 * C].reshape(NCH, P).T)
    bo_col = np.ascontiguousarray(np.asarray(out_b, dtype=np.float32).reshape(NCH, P).T)
    bp_col = np.ascontiguousarray(np.asarray(proj_b, dtype=np.float32).reshape(NCH, P).T)
    gamma_col = np.full((P, 1), np.asarray(gamma, dtype=np.float32).reshape(-1)[0],
                        dtype=np.float32)

    vis2d = visual.reshape(B, C, HW)
    in_maps = []
    for c in range(B):
        in_maps.append({
            "vis": vis2d[c],
            "textT": np.ascontiguousarray(text[c].T),
            "wvT": wvT, "woT": woT, "wpT": wpT,
            "bv_col": bv_col, "bo_col": bo_col, "bp_col": bp_col,
            "gamma_col": gamma_col,
        })

    nc = _get_nc()
    trace = os.environ.get("BASS_KERNEL_TRACE", "") == "1"
    if trace:
        _install_ntff_hook()
    res = run_bass_kernel_spmd(nc, in_maps, core_ids=list(range(B)), trace=trace)
    if trace:
        _NC_CACHE["last_results"] = res

    out = np.empty((B, C, HW), dtype=np.float32)
    for c in range(B):
        out[c] = res.results[c]["out"]
    return out.reshape(B, C, H, W)


# revision 6
# speedup vs baseline: 1.3051x; 1.3051x over previous
"""Trainium2 Bass kernel for CrossModalAttention2D.

Math note: the attention has kv_len == 1 (text is mean-pooled to a single
token), so softmax over the key axis is identically 1.0 and the attention
output for every query position equals v[b].  The LayerNorm + Q projection
therefore do not affect the output at all; the module reduces exactly to

    t[b]   = mean_n text[b, n, :]                      # (C,)
    p[b]   = ((t Wv^T + bv) out_w^T + out_b) proj_w^T + proj_b
    out    = visual + gamma * p[b][None, :, None, None]

which is what this kernel computes (verified to 5e-8 normalized rel err
against the full reference).  Sharding: data-parallel over B — core c
handles batch c.  Weights (3 x 1024x1024) are replicated to every core and
the small chain is recomputed per core; the 16 MB visual slice per core
dominates traffic.
"""

import os
import sys

sys.path.insert(0, "/opt/trn_rl_repo")

import numpy as np

import concourse.bass as bass
import concourse.mybir as mybir
from concourse.tile import TileContext
from concourse.bass_utils import run_bass_kernel_spmd

B, C, H, W, NH, NT = 8, 1024, 64, 64, 16, 8
HW = H * W
P = 128
NCH = C // P  # 8 channel chunks
F32 = mybir.dt.float32


def _split_waits(nc):
    """walrus in this env accepts at most ONE sync-wait per instruction.
    Hoist extra waits onto NoOps inserted just before, on the same engine
    (per-engine program order makes this semantically identical)."""
    for fn in nc.m.functions:
        for blk in fn.blocks:
            rebuilt = []
            changed = False
            for inst in blk.instructions:
                si = inst.sync_info
                if si is not None and si.on_wait is not None and len(si.on_wait) > 1:
                    waits = list(si.on_wait)
                    for i, w in enumerate(waits[:-1]):
                        rebuilt.append(
                            mybir.InstNoOp(
                                name=f"{inst.name}-ws{i}",
                                engine=inst.engine,
                                sync_info=mybir.SyncInfo(on_wait=[w], on_update=[]),
                                bass_nofuse=True,
                            )
                        )
                    si.on_wait = [waits[-1]]
                    changed = True
                rebuilt.append(inst)
            if changed:
                blk.instructions = rebuilt


BF16 = mybir.dt.bfloat16


def _build_nc(vis_bufs=8, w_bufs=4):
    nc = bass.Bass(trn_type="TRN2")

    vis = nc.dram_tensor("vis", [C, HW], F32, kind="ExternalInput")
    textT = nc.dram_tensor("textT", [C, NT], F32, kind="ExternalInput")
    # chain weights in bf16: 4x cheaper on the PE (1-pass LDWEIGHTS) and
    # half the DMA bytes; the ~1e-3 relative error on the tiny broadcast
    # vector p is invisible next to visual (checked: ~2e-5 output rel err)
    wvT = nc.dram_tensor("wvT", [C, C], BF16, kind="ExternalInput")
    woT = nc.dram_tensor("woT", [C, C], BF16, kind="ExternalInput")
    wpT = nc.dram_tensor("wpT", [C, C], BF16, kind="ExternalInput")
    bv_col = nc.dram_tensor("bv_col", [P, NCH], F32, kind="ExternalInput")
    bo_col = nc.dram_tensor("bo_col", [P, NCH], F32, kind="ExternalInput")
    bp_col = nc.dram_tensor("bp_col", [P, NCH], F32, kind="ExternalInput")
    gamma_col = nc.dram_tensor("gamma_col", [P, 1], F32, kind="ExternalInput")
    out = nc.dram_tensor("out", [C, HW], F32, kind="ExternalOutput")

    with TileContext(nc) as tc:
        with (
            tc.tile_pool(name="cols", bufs=1) as cols,
            tc.tile_pool(name="wpool", bufs=w_bufs) as wpool,
            tc.tile_pool(name="psum", bufs=NCH, space="PSUM") as psum,
            tc.tile_pool(name="vispool", bufs=vis_bufs) as vispool,
        ):
            # ---- constants (SWDGE queue; HW queues are reserved for vis) ----
            bvc = cols.tile([P, NCH], F32, tag="bvc")
            boc = cols.tile([P, NCH], F32, tag="boc")
            bpc = cols.tile([P, NCH], F32, tag="bpc")
            gc = cols.tile([P, 1], F32, tag="gc")
            nc.gpsimd.dma_start(out=bvc, in_=bv_col[:, :])
            nc.gpsimd.dma_start(out=boc, in_=bo_col[:, :])
            nc.gpsimd.dma_start(out=bpc, in_=bp_col[:, :])
            nc.gpsimd.dma_start(out=gc, in_=gamma_col[:, :])

            # ---- t = mean(text) in column layout: NCH tiles of (P, 1) ----
            tcols = []
            for k in range(NCH):
                tt = cols.tile([P, NT], F32, tag=f"tt{k}")
                nc.gpsimd.dma_start(out=tt, in_=textT[k * P : (k + 1) * P, :])
                tk = cols.tile([P, 1], F32, tag=f"t{k}")
                nc.vector.reduce_sum(tk, tt, axis=mybir.AxisListType.X)
                nc.vector.tensor_scalar_mul(tk, tk, 1.0 / NT)
                tkb = cols.tile([P, 1], BF16, tag=f"tb{k}")
                nc.vector.tensor_copy(tkb, tk)
                tcols.append(tkb)

            # ---- 3-layer chain: each layer out_col[mo] = sum_k WT[k-chunk, mo-chunk]^T @ in_col[k] ----
            def layer(in_cols, wT_dram, bias_tile, name, gamma_tile=None):
                ps = [psum.tile([P, 1], F32, tag="ps", name=f"ps_{name}{i}")
                      for i in range(NCH)]
                for k in range(NCH):
                    wt = wpool.tile([P, C], BF16, tag="wt")
                    nc.gpsimd.dma_start(out=wt, in_=wT_dram[k * P : (k + 1) * P, :])
                    for mo in range(NCH):
                        nc.tensor.matmul(
                            ps[mo],
                            wt[:, mo * P : (mo + 1) * P],
                            in_cols[k],
                            start=(k == 0),
                            stop=(k == NCH - 1),
                        )
                outs = []
                for mo in range(NCH):
                    last = name == "g"
                    oc = cols.tile([P, 1], F32 if last else BF16, tag=f"{name}{mo}")
                    if gamma_tile is None:
                        nc.vector.tensor_tensor(
                            out=oc, in0=ps[mo], in1=bias_tile[:, mo : mo + 1],
                            op=mybir.AluOpType.add,
                        )
                    else:
                        # oc = (psum + bias) * gamma
                        nc.vector.tensor_scalar(
                            oc, ps[mo],
                            bias_tile[:, mo : mo + 1], gamma_tile[:, 0:1],
                            op0=mybir.AluOpType.add, op1=mybir.AluOpType.mult,
                        )
                    outs.append(oc)
                return outs

            vcols = layer(tcols, wvT, bvc, "v")
            ucols = layer(vcols, woT, boc, "u")
            gpcols = layer(ucols, wpT, bpc, "g", gamma_tile=gc)

            # ---- main loop: out = vis + gp (broadcast along free dim) ----
            # split visual traffic across both HWDGE queues (SP + Activation)
            for m in range(NCH):
                in_eng = nc.sync if m % 2 == 0 else nc.scalar
                out_eng = nc.scalar if m % 2 == 0 else nc.sync
                vt = vispool.tile([P, HW], F32, tag="vt")
                in_eng.dma_start(out=vt, in_=vis[m * P : (m + 1) * P, :])
                nc.vector.tensor_scalar_add(vt, vt, gpcols[m][:, 0:1])
                out_eng.dma_start(out=out[m * P : (m + 1) * P, :], in_=vt)

    _split_waits(nc)
    return nc


def _install_ntff_hook():
    """This container's `antenv` stub lacks axon_hooks; recreate the NTFF
    profiling hook via ctypes against the axon PJRT .so (same logic as
    trn_agent_boot.trn_boot)."""
    try:
        from antenv.axon_hooks import get_axon_ntff_profile_hook  # noqa: F401
        return
    except ImportError:
        pass
    import contextlib
    import ctypes
    import types

    so_path = "/opt/axon/libaxon_pjrt.so"
    if not os.path.exists(so_path):
        return
    lib = ctypes.CDLL(so_path)
    if not hasattr(lib, "axon_start_nrt_profile"):
        return
    lib.axon_start_nrt_profile.argtypes = [
        ctypes.POINTER(ctypes.c_int64), ctypes.c_size_t,
    ]
    lib.axon_start_nrt_profile.restype = ctypes.c_int64
    lib.axon_stop_nrt_profile.argtypes = [ctypes.c_char_p]
    lib.axon_stop_nrt_profile.restype = ctypes.c_int64

    @contextlib.contextmanager
    def _hook(output_dir, device_ids):
        import jax

        jax.devices()
        if device_ids:
            ids = (ctypes.c_int64 * len(device_ids))(*device_ids)
            rc = lib.axon_start_nrt_profile(ids, len(device_ids))
        else:
            rc = lib.axon_start_nrt_profile(None, 0)
        if rc != 0:
            raise RuntimeError(f"axon_start_nrt_profile rc={rc}")
        try:
            yield
        finally:
            n = lib.axon_stop_nrt_profile(str(output_dir).encode())
            print(f"ntff profile: {n} file(s) written to {output_dir}")

    import antenv

    mod = types.ModuleType("antenv.axon_hooks")
    mod.get_axon_ntff_profile_hook = lambda: _hook
    mod.set_axon_ntff_profile_hook = lambda h: None
    sys.modules["antenv.axon_hooks"] = mod
    antenv.axon_hooks = mod


_NC_CACHE = {}


def _get_nc():
    if "nc" not in _NC_CACHE:
        _NC_CACHE["nc"] = _build_nc()
    return _NC_CACHE["nc"]


def kernel(visual, text, in_proj_w, in_proj_b, out_w, out_b, ln_w, ln_b,
           proj_w, proj_b, gamma):
    visual = np.ascontiguousarray(np.asarray(visual, dtype=np.float32))
    text = np.asarray(text, dtype=np.float32)
    in_proj_w = np.asarray(in_proj_w, dtype=np.float32)
    in_proj_b = np.asarray(in_proj_b, dtype=np.float32)

    # host-side input marshalling (layout/dtype only, no math)
    import ml_dtypes

    bf16 = ml_dtypes.bfloat16
    wvT = np.ascontiguousarray(in_proj_w[2 * C : 3 * C].T).astype(bf16)   # [c, j]
    woT = np.ascontiguousarray(np.asarray(out_w, dtype=np.float32).T).astype(bf16)
    wpT = np.ascontiguousarray(np.asarray(proj_w, dtype=np.float32).T).astype(bf16)
    bv_col = np.ascontiguousarray(in_proj_b[2 * C : 3 * C].reshape(NCH, P).T)
    bo_col = np.ascontiguousarray(np.asarray(out_b, dtype=np.float32).reshape(NCH, P).T)
    bp_col = np.ascontiguousarray(np.asarray(proj_b, dtype=np.float32).reshape(NCH, P).T)
    gamma_col = np.full((P, 1), np.asarray(gamma, dtype=np.float32).reshape(-1)[0],
                        dtype=np.float32)

    vis2d = visual.reshape(B, C, HW)
    in_maps = []
    for c in range(B):
        in_maps.append({
            "vis": vis2d[c],
            "textT": np.ascontiguousarray(text[c].T),
            "wvT": wvT, "woT": woT, "wpT": wpT,
            "bv_col": bv_col, "bo_col": bo_col, "bp_col": bp_col,
            "gamma_col": gamma_col,
        })

    nc = _get_nc()
    trace = os.environ.get("BASS_KERNEL_TRACE", "") == "1"
    if trace:
        _install_ntff_hook()
    res = run_bass_kernel_spmd(nc, in_maps, core_ids=list(range(B)), trace=trace)
    if trace:
        _NC_CACHE["last_results"] = res

    out = np.empty((B, C, HW), dtype=np.float32)
    for c in range(B):
        out[c] = res.results[c]["out"]
    return out.reshape(B, C, H, W)


# revision 8
# speedup vs baseline: 1.3780x; 1.0559x over previous
"""Trainium2 Bass kernel for CrossModalAttention2D.

Math note: the attention has kv_len == 1 (text is mean-pooled to a single
token), so softmax over the key axis is identically 1.0 and the attention
output for every query position equals v[b].  The LayerNorm + Q projection
therefore do not affect the output at all; the module reduces exactly to

    t[b]   = mean_n text[b, n, :]                      # (C,)
    p[b]   = ((t Wv^T + bv) out_w^T + out_b) proj_w^T + proj_b
    out    = visual + gamma * p[b][None, :, None, None]

which is what this kernel computes (verified to 5e-8 normalized rel err
against the full reference).  Sharding: data-parallel over B — core c
handles batch c.  Weights (3 x 1024x1024) are replicated to every core and
the small chain is recomputed per core; the 16 MB visual slice per core
dominates traffic.
"""

import os
import sys

sys.path.insert(0, "/opt/trn_rl_repo")

import numpy as np

import concourse.bass as bass
import concourse.mybir as mybir
from concourse.tile import TileContext
from concourse.bass_utils import run_bass_kernel_spmd

B, C, H, W, NH, NT = 8, 1024, 64, 64, 16, 8
HW = H * W
P = 128
NCH = C // P  # 8 channel chunks
F32 = mybir.dt.float32


def _split_waits(nc):
    """walrus in this env accepts at most ONE sync-wait per instruction.
    Hoist extra waits onto NoOps inserted just before, on the same engine
    (per-engine program order makes this semantically identical)."""
    for fn in nc.m.functions:
        for blk in fn.blocks:
            rebuilt = []
            changed = False
            for inst in blk.instructions:
                si = inst.sync_info
                if si is not None and si.on_wait is not None and len(si.on_wait) > 1:
                    waits = list(si.on_wait)
                    for i, w in enumerate(waits[:-1]):
                        rebuilt.append(
                            mybir.InstNoOp(
                                name=f"{inst.name}-ws{i}",
                                engine=inst.engine,
                                sync_info=mybir.SyncInfo(on_wait=[w], on_update=[]),
                                bass_nofuse=True,
                            )
                        )
                    si.on_wait = [waits[-1]]
                    changed = True
                rebuilt.append(inst)
            if changed:
                blk.instructions = rebuilt


BF16 = mybir.dt.bfloat16


def _build_nc(vis_bufs=8, w_bufs=4):
    nc = bass.Bass(trn_type="TRN2")

    vis = nc.dram_tensor("vis", [C, HW], F32, kind="ExternalInput")
    textT = nc.dram_tensor("textT", [C, NT], F32, kind="ExternalInput")
    # chain weights in bf16: 4x cheaper on the PE (1-pass LDWEIGHTS) and
    # half the DMA bytes; the ~1e-3 relative error on the tiny broadcast
    # vector p is invisible next to visual (checked: ~2e-5 output rel err)
    wvT = nc.dram_tensor("wvT", [C, C], BF16, kind="ExternalInput")
    woT = nc.dram_tensor("woT", [C, C], BF16, kind="ExternalInput")
    wpT = nc.dram_tensor("wpT", [C, C], BF16, kind="ExternalInput")
    bv_col = nc.dram_tensor("bv_col", [P, NCH], F32, kind="ExternalInput")
    bo_col = nc.dram_tensor("bo_col", [P, NCH], F32, kind="ExternalInput")
    bp_col = nc.dram_tensor("bp_col", [P, NCH], F32, kind="ExternalInput")
    gamma_col = nc.dram_tensor("gamma_col", [P, 1], F32, kind="ExternalInput")
    out = nc.dram_tensor("out", [C, HW], F32, kind="ExternalOutput")

    with TileContext(nc) as tc:
        with (
            tc.tile_pool(name="cols", bufs=1) as cols,
            tc.tile_pool(name="wpool", bufs=w_bufs) as wpool,
            tc.tile_pool(name="psum", bufs=NCH, space="PSUM") as psum,
            tc.tile_pool(name="vispool", bufs=vis_bufs) as vispool,
        ):
            # ---- constants (SWDGE queue; HW queues are reserved for vis) ----
            bvc = cols.tile([P, NCH], F32, tag="bvc")
            boc = cols.tile([P, NCH], F32, tag="boc")
            bpc = cols.tile([P, NCH], F32, tag="bpc")
            gc = cols.tile([P, 1], F32, tag="gc")
            nc.gpsimd.dma_start(out=bvc, in_=bv_col[:, :])
            nc.gpsimd.dma_start(out=boc, in_=bo_col[:, :])
            nc.gpsimd.dma_start(out=bpc, in_=bp_col[:, :])
            nc.gpsimd.dma_start(out=gc, in_=gamma_col[:, :])

            # ---- t = mean(text) in column layout: NCH tiles of (P, 1) ----
            tcols = []
            for k in range(NCH):
                tt = cols.tile([P, NT], F32, tag=f"tt{k}")
                nc.gpsimd.dma_start(out=tt, in_=textT[k * P : (k + 1) * P, :])
                tk = cols.tile([P, 1], F32, tag=f"t{k}")
                nc.vector.reduce_sum(tk, tt, axis=mybir.AxisListType.X)
                nc.vector.tensor_scalar_mul(tk, tk, 1.0 / NT)
                tkb = cols.tile([P, 1], BF16, tag=f"tb{k}")
                nc.vector.tensor_copy(tkb, tk)
                tcols.append(tkb)

            # ---- 3-layer chain: each layer out_col[mo] = sum_k WT[k-chunk, mo-chunk]^T @ in_col[k] ----
            # weights ride the fast HW queues (issued first, so the chain —
            # which gates every output store — completes early)
            def layer(in_cols, wT_dram, bias_tile, name, gamma_tile=None):
                ps = [psum.tile([P, 1], F32, tag="ps", name=f"ps_{name}{i}")
                      for i in range(NCH)]
                for k in range(NCH):
                    wt = wpool.tile([P, C], BF16, tag="wt")
                    w_eng = nc.sync if k % 2 == 0 else nc.scalar
                    w_eng.dma_start(out=wt, in_=wT_dram[k * P : (k + 1) * P, :])
                    for mo in range(NCH):
                        nc.tensor.matmul(
                            ps[mo],
                            wt[:, mo * P : (mo + 1) * P],
                            in_cols[k],
                            start=(k == 0),
                            stop=(k == NCH - 1),
                        )
                outs = []
                for mo in range(NCH):
                    last = name == "g"
                    oc = cols.tile([P, 1], F32 if last else BF16, tag=f"{name}{mo}")
                    if gamma_tile is None:
                        nc.vector.tensor_tensor(
                            out=oc, in0=ps[mo], in1=bias_tile[:, mo : mo + 1],
                            op=mybir.AluOpType.add,
                        )
                    else:
                        # oc = (psum + bias) * gamma
                        nc.vector.tensor_scalar(
                            oc, ps[mo],
                            bias_tile[:, mo : mo + 1], gamma_tile[:, 0:1],
                            op0=mybir.AluOpType.add, op1=mybir.AluOpType.mult,
                        )
                    outs.append(oc)
                return outs

            vcols = layer(tcols, wvT, bvc, "v")
            ucols = layer(vcols, woT, boc, "u")
            gpcols = layer(ucols, wpT, bpc, "g", gamma_tile=gc)

            # ---- main loop: out = vis + gp (broadcast along free dim) ----
            # visual traffic rides all three DMA paths: both HWDGE queues
            # (SP + Activation) plus the SWDGE queue (GpSimd), which adds
            # bandwidth on top of the HW-DGE cluster cap
            for m in range(NCH):
                if m >= 6:
                    in_eng = out_eng = nc.gpsimd
                else:
                    in_eng = nc.sync if m % 2 == 0 else nc.scalar
                    out_eng = nc.scalar if m % 2 == 0 else nc.sync
                vt = vispool.tile([P, HW], F32, tag="vt")
                in_eng.dma_start(out=vt, in_=vis[m * P : (m + 1) * P, :])
                nc.vector.tensor_scalar_add(vt, vt, gpcols[m][:, 0:1])
                out_eng.dma_start(out=out[m * P : (m + 1) * P, :], in_=vt)

    _split_waits(nc)
    return nc


def _install_ntff_hook():
    """This container's `antenv` stub lacks axon_hooks; recreate the NTFF
    profiling hook via ctypes against the axon PJRT .so (same logic as
    trn_agent_boot.trn_boot)."""
    try:
        from antenv.axon_hooks import get_axon_ntff_profile_hook  # noqa: F401
        return
    except ImportError:
        pass
    import contextlib
    import ctypes
    import types

    so_path = "/opt/axon/libaxon_pjrt.so"
    if not os.path.exists(so_path):
        return
    lib = ctypes.CDLL(so_path)
    if not hasattr(lib, "axon_start_nrt_profile"):
        return
    lib.axon_start_nrt_profile.argtypes = [
        ctypes.POINTER(ctypes.c_int64), ctypes.c_size_t,
    ]
    lib.axon_start_nrt_profile.restype = ctypes.c_int64
    lib.axon_stop_nrt_profile.argtypes = [ctypes.c_char_p]
    lib.axon_stop_nrt_profile.restype = ctypes.c_int64

    @contextlib.contextmanager
    def _hook(output_dir, device_ids):
        import jax

        jax.devices()
        if device_ids:
            ids = (ctypes.c_int64 * len(device_ids))(*device_ids)
            rc = lib.axon_start_nrt_profile(ids, len(device_ids))
        else:
            rc = lib.axon_start_nrt_profile(None, 0)
        if rc != 0:
            raise RuntimeError(f"axon_start_nrt_profile rc={rc}")
        try:
            yield
        finally:
            n = lib.axon_stop_nrt_profile(str(output_dir).encode())
            print(f"ntff profile: {n} file(s) written to {output_dir}")

    import antenv

    mod = types.ModuleType("antenv.axon_hooks")
    mod.get_axon_ntff_profile_hook = lambda: _hook
    mod.set_axon_ntff_profile_hook = lambda h: None
    sys.modules["antenv.axon_hooks"] = mod
    antenv.axon_hooks = mod


_NC_CACHE = {}


def _get_nc():
    if "nc" not in _NC_CACHE:
        _NC_CACHE["nc"] = _build_nc()
    return _NC_CACHE["nc"]


def kernel(visual, text, in_proj_w, in_proj_b, out_w, out_b, ln_w, ln_b,
           proj_w, proj_b, gamma):
    visual = np.ascontiguousarray(np.asarray(visual, dtype=np.float32))
    text = np.asarray(text, dtype=np.float32)
    in_proj_w = np.asarray(in_proj_w, dtype=np.float32)
    in_proj_b = np.asarray(in_proj_b, dtype=np.float32)

    # host-side input marshalling (layout/dtype only, no math)
    import ml_dtypes

    bf16 = ml_dtypes.bfloat16
    wvT = np.ascontiguousarray(in_proj_w[2 * C : 3 * C].T).astype(bf16)   # [c, j]
    woT = np.ascontiguousarray(np.asarray(out_w, dtype=np.float32).T).astype(bf16)
    wpT = np.ascontiguousarray(np.asarray(proj_w, dtype=np.float32).T).astype(bf16)
    bv_col = np.ascontiguousarray(in_proj_b[2 * C : 3 * C].reshape(NCH, P).T)
    bo_col = np.ascontiguousarray(np.asarray(out_b, dtype=np.float32).reshape(NCH, P).T)
    bp_col = np.ascontiguousarray(np.asarray(proj_b, dtype=np.float32).reshape(NCH, P).T)
    gamma_col = np.full((P, 1), np.asarray(gamma, dtype=np.float32).reshape(-1)[0],
                        dtype=np.float32)

    vis2d = visual.reshape(B, C, HW)
    in_maps = []
    for c in range(B):
        in_maps.append({
            "vis": vis2d[c],
            "textT": np.ascontiguousarray(text[c].T),
            "wvT": wvT, "woT": woT, "wpT": wpT,
            "bv_col": bv_col, "bo_col": bo_col, "bp_col": bp_col,
            "gamma_col": gamma_col,
        })

    nc = _get_nc()
    trace = os.environ.get("BASS_KERNEL_TRACE", "") == "1"
    if trace:
        _install_ntff_hook()
    res = run_bass_kernel_spmd(nc, in_maps, core_ids=list(range(B)), trace=trace)
    if trace:
        _NC_CACHE["last_results"] = res

    out = np.empty((B, C, HW), dtype=np.float32)
    for c in range(B):
        out[c] = res.results[c]["out"]
    return out.reshape(B, C, H, W)
